# revision 15
# baseline (speedup 1.0000x reference)
"""Batched compressed linear v2: y = x @ (w_q * scale).T + bias on 8 TRN2 cores.

Sharding: column-parallel over out_features (16384 -> 8 x 2048).
Each core computes y_shard[8192, 2048] = x[8192, 4096] @ wT_shard + bias_shard.

v2 changes vs baseline:
  - scale folded into w at preproc (w_sc = bf16(w_q) * scale, one-time)
    -> psum evict is a single DVE add of bias.
  - per-m-tile x cast chunks (64 x 2MB SWDGE) instead of 16 x 12.6MB,
    so transposes unblock early.
  - w preproc chunked [128, 1024] and emitted first (PE warms up ~30us in).
  - k-outer / ob-inner main loop: 4 matmuls share one stationary xT tile;
    post-compile LDWEIGHTS dedup removes the 3 redundant weight loads.
"""

import sys

if "/opt/trn_rl_repo" not in sys.path:
    sys.path.insert(0, "/opt/trn_rl_repo")

import numpy as np

B, S, IN_F, OUT_F = 4, 2048, 4096, 16384
NCORES = 8
O_SHARD = OUT_F // NCORES  # 2048
M_FULL = B * S  # 8192


def dedup_ldweights(nc):
    """Remove back-to-back InstLdweights with identical operands and no
    semaphore waits. The PE array keeps the stationary operand across
    matmuls, so a reload of the same weights is pure overhead (~107ns)."""
    import concourse.mybir as mybir

    total_removed = 0
    for blk in nc.m.functions[0].blocks:
        insts = list(blk.instructions)
        new = []
        last_key = None
        changed = False
        for i in insts:
            tn = type(i).__name__
            if tn == "InstLdweights":
                c = i.concise()
                key = (c[c.find("in=") :], str(i.tile_size), str(i.tile_position))
                has_wait = "wait:" in c
                if key == last_key and not has_wait:
                    total_removed += 1
                    changed = True
                    continue
                last_key = key
            elif tn in ("InstMatmult", "InstEventSemaphore"):
                pass  # don't invalidate the loaded stationary
            else:
                if getattr(i, "engine", None) == mybir.EngineType.PE:
                    last_key = None
            new.append(i)
        if changed:
            blk.instructions = new
    return total_removed


def build_kernel_v2(nc, tc, M, K, O):
    import concourse.mybir as mybir

    f32 = mybir.dt.float32
    bf16 = mybir.dt.bfloat16
    i32 = mybir.dt.int32

    assert M % 128 == 0 and K % 1024 == 0 and O % 512 == 0
    KT = K // 128  # 32 contraction tiles
    MT = M // 128  # 64 m tiles
    NB = O // 512  # 4 psum-bank columns
    OT = O // 128  # 16 w row-chunks
    WCH = 1024  # w free-dim chunk for staging
    NWQ = K // WCH  # 4 chunks per ot

    x_d = nc.dram_tensor("x", [M, K], f32, kind="ExternalInput").ap()
    w_d = nc.dram_tensor("w_q", [O, K], i32, kind="ExternalInput").ap()
    scale_d = nc.dram_tensor("scale", [1], f32, kind="ExternalInput").ap()
    bias_d = nc.dram_tensor("bias", [O], f32, kind="ExternalInput").ap()
    y_d = nc.dram_tensor("y", [M, O], f32, kind="ExternalOutput").ap()

    from contextlib import ExitStack

    ctx = ExitStack()
    tc_pool = lambda **kw: ctx.enter_context(tc.tile_pool(**kw))

    consts = tc_pool(name="consts", bufs=1)
    wq_pool = tc_pool(name="wq", bufs=2)
    xt_pool = tc_pool(name="xt", bufs=4)
    out_pool = tc_pool(name="outsb", bufs=6)
    psum_pool = tc_pool(name="psum", bufs=2, space="PSUM")
    dram_pool = tc_pool(name="dram", bufs=1, space="DRAM")

    # ---- constants ----
    scale128 = consts.tile([128, 1], f32, tag="scale128")
    nc.sync.dma_start(scale128[:], scale_d[None, :].partition_broadcast(128))
    bias_bcast = consts.tile([128, O], f32, tag="bias_bcast")
    nc.sync.dma_start(bias_bcast[:], bias_d[None, :].partition_broadcast(128))

    NPRE = 3  # x tiles pre-transposed ahead of the w preproc

    # ---- x cast: fp32 -> bf16 DRAM scratch (SWDGE), per m-tile.
    # First NPRE casts lead the gpsimd ring; w loads q2/q3 follow; the
    # remaining casts trail (they only need to stay ~4 tiles ahead of PE).
    x_bf_d = dram_pool.tile([M, K], bf16, tag="x_bf", name="x_bf")

    def emit_cast(mt):
        nc.gpsimd.dma_start(
            x_bf_d[mt * 128 : (mt + 1) * 128, :], x_d[mt * 128 : (mt + 1) * 128, :]
        )

    def emit_xt(mt):
        # xT transposes ride the sync ring (scalar ring is w-transpose only)
        xT = xt_pool.tile([128, KT, 128], bf16, tag="xT", name=f"xT{mt}")
        nc.sync.dma_start(
            xT[:], x_bf_d[mt * 128 : (mt + 1) * 128, :], transpose=True
        )
        return xT

    for mt in range(NPRE):
        emit_cast(mt)
    xt_q = [emit_xt(mt) for mt in range(NPRE)]

    # ---- w preproc: int32 -> bf16 * scale -> SBUF->SBUF xbar transpose ----
    # wT_banks[b][p, k, j] = w_sc[b*512 + j, k*128 + p]
    wT_banks = [
        consts.tile([128, KT, 512], bf16, tag=f"wT{b}", name=f"wT{b}")
        for b in range(NB)
    ]
    for ot in range(OT):
        r0 = ot * 128
        w_sc4 = wq_pool.tile([128, K], bf16, tag="wsc4")
        for q in range(NWQ):
            c0 = q * WCH
            w_stage = wq_pool.tile([128, WCH], i32, tag="wstage")
            eng = nc.sync if q < 2 else nc.gpsimd
            eng.dma_start(w_stage[:], w_d[r0 : r0 + 128, c0 : c0 + WCH])
            nc.vector.tensor_scalar_mul(
                w_sc4[:, c0 : c0 + WCH], w_stage[:], scale128[:, 0:1]
            )
        b, col = ot // 4, (ot % 4) * 128
        nc.scalar.dma_start(
            wT_banks[b][:, :, col : col + 128], w_sc4[:], transpose=True
        )

    for mt in range(NPRE, MT):
        emit_cast(mt)

    # ---- main loop ----
    for mt in range(MT):
        if xt_q:
            xT = xt_q.pop(0)
        else:
            xT = emit_xt(mt)
        banks = [
            psum_pool.tile([128, 512], f32, tag=f"ps{b}", name=f"ps{mt}_{b}")
            for b in range(NB)
        ]
        for k in range(KT):
            for ob in range(NB):
                nc.tensor.matmul(
                    banks[ob][:],
                    xT[:, k, :],
                    wT_banks[ob][:, k, :],
                    start=(k == 0),
                    stop=(k == KT - 1),
                )
        for ob in range(NB):
            out_sb = out_pool.tile([128, 512], f32, tag="out", name=f"out{mt}_{ob}")
            nc.vector.tensor_add(
                out_sb[:], banks[ob][:], bias_bcast[:, ob * 512 : (ob + 1) * 512]
            )
            nc.sync.dma_start(
                y_d[mt * 128 : (mt + 1) * 128, ob * 512 : (ob + 1) * 512], out_sb[:]
            )

    ctx.close()


_CACHED_NC = None
LAST_RESULT = None


def _build_full_nc():
    global _CACHED_NC
    if _CACHED_NC is not None:
        return _CACHED_NC
    import concourse.tile as tile
    from concourse import bacc

    nc = bacc.Bacc(
        "TRN2",
        target_bir_lowering=False,
        debug=False,
        num_devices=NCORES,
    )
    with tile.TileContext(nc) as tc:
        build_kernel_v2(nc, tc, M_FULL, IN_F, O_SHARD)
    nc.compile()
    import os

    # NOTE: LDWEIGHTS dedup measured 3x SLOWER on HW (829 vs 278 ns/MM) --
    # the PE path needs the LDW/MM pairing. Keep redundant LDWs.
    if os.environ.get("DO_DEDUP") == "1":
        n = dedup_ldweights(nc)
        print(f"dedup_ldweights removed {n} instructions", file=sys.stderr)
    _CACHED_NC = nc
    return nc


def kernel(x, w_q, scale, bias):
    """Full inputs in, full output out. Shards w_q/bias over 8 cores."""
    from concourse.bass_utils import run_bass_kernel_spmd

    nc = _build_full_nc()

    x2 = np.ascontiguousarray(np.asarray(x, dtype=np.float32).reshape(M_FULL, IN_F))
    w2 = np.ascontiguousarray(np.asarray(w_q, dtype=np.int32))
    sc = np.asarray(scale, dtype=np.float32).reshape(1)
    bi = np.asarray(bias, dtype=np.float32)

    in_maps = []
    for c in range(NCORES):
        o0 = c * O_SHARD
        in_maps.append(
            {
                "x": x2,
                "w_q": np.ascontiguousarray(w2[o0 : o0 + O_SHARD]),
                "scale": sc,
                "bias": np.ascontiguousarray(bi[o0 : o0 + O_SHARD]),
            }
        )

    res = run_bass_kernel_spmd(nc, in_maps, core_ids=list(range(NCORES)))
    global LAST_RESULT
    LAST_RESULT = res
    shards = [res.results[c]["y"] for c in range(NCORES)]
    y = np.concatenate(shards, axis=1).reshape(B, S, OUT_F)
    return y.astype(np.float32)


# revision 16
# speedup vs baseline: 1.0040x; 1.0040x over previous
"""Batched compressed linear v2: y = x @ (w_q * scale).T + bias on 8 TRN2 cores.

Sharding: column-parallel over out_features (16384 -> 8 x 2048).
Each core computes y_shard[8192, 2048] = x[8192, 4096] @ wT_shard + bias_shard.

v2 changes vs baseline:
  - scale folded into w at preproc (w_sc = bf16(w_q) * scale, one-time)
    -> psum evict is a single DVE add of bias.
  - per-m-tile x cast chunks (64 x 2MB SWDGE) instead of 16 x 12.6MB,
    so transposes unblock early.
  - w preproc chunked [128, 1024] and emitted first (PE warms up ~30us in).
  - k-outer / ob-inner main loop: 4 matmuls share one stationary xT tile;
    post-compile LDWEIGHTS dedup removes the 3 redundant weight loads.
"""

import sys

if "/opt/trn_rl_repo" not in sys.path:
    sys.path.insert(0, "/opt/trn_rl_repo")

import numpy as np

B, S, IN_F, OUT_F = 4, 2048, 4096, 16384
NCORES = 8
O_SHARD = OUT_F // NCORES  # 2048
M_FULL = B * S  # 8192


def dedup_ldweights(nc):
    """Remove back-to-back InstLdweights with identical operands and no
    semaphore waits. The PE array keeps the stationary operand across
    matmuls, so a reload of the same weights is pure overhead (~107ns)."""
    import concourse.mybir as mybir

    total_removed = 0
    for blk in nc.m.functions[0].blocks:
        insts = list(blk.instructions)
        new = []
        last_key = None
        changed = False
        for i in insts:
            tn = type(i).__name__
            if tn == "InstLdweights":
                c = i.concise()
                key = (c[c.find("in=") :], str(i.tile_size), str(i.tile_position))
                has_wait = "wait:" in c
                if key == last_key and not has_wait:
                    total_removed += 1
                    changed = True
                    continue
                last_key = key
            elif tn in ("InstMatmult", "InstEventSemaphore"):
                pass  # don't invalidate the loaded stationary
            else:
                if getattr(i, "engine", None) == mybir.EngineType.PE:
                    last_key = None
            new.append(i)
        if changed:
            blk.instructions = new
    return total_removed


def build_kernel_v2(nc, tc, M, K, O):
    import concourse.mybir as mybir

    f32 = mybir.dt.float32
    bf16 = mybir.dt.bfloat16
    i32 = mybir.dt.int32

    assert M % 128 == 0 and K % 1024 == 0 and O % 512 == 0
    KT = K // 128  # 32 contraction tiles
    MT = M // 128  # 64 m tiles
    NB = O // 512  # 4 psum-bank columns
    OT = O // 128  # 16 w row-chunks
    WCH = 1024  # w free-dim chunk for staging
    NWQ = K // WCH  # 4 chunks per ot

    x_d = nc.dram_tensor("x", [M, K], f32, kind="ExternalInput").ap()
    w_d = nc.dram_tensor("w_q", [O, K], i32, kind="ExternalInput").ap()
    scale_d = nc.dram_tensor("scale", [1], f32, kind="ExternalInput").ap()
    bias_d = nc.dram_tensor("bias", [O], f32, kind="ExternalInput").ap()
    y_d = nc.dram_tensor("y", [M, O], f32, kind="ExternalOutput").ap()

    from contextlib import ExitStack

    ctx = ExitStack()
    tc_pool = lambda **kw: ctx.enter_context(tc.tile_pool(**kw))

    consts = tc_pool(name="consts", bufs=1)
    wq_pool = tc_pool(name="wq", bufs=2)
    xt_pool = tc_pool(name="xt", bufs=3)
    out_pool = tc_pool(name="outsb", bufs=4)
    psum_pool = tc_pool(name="psum", bufs=2, space="PSUM")
    dram_pool = tc_pool(name="dram", bufs=1, space="DRAM")

    # ---- constants ----
    scale128 = consts.tile([128, 1], f32, tag="scale128")
    nc.sync.dma_start(scale128[:], scale_d[None, :].partition_broadcast(128))
    bias_bcast = consts.tile([128, O], f32, tag="bias_bcast")
    nc.sync.dma_start(bias_bcast[:], bias_d[None, :].partition_broadcast(128))

    NPRE = 3  # x tiles pre-transposed ahead of the w preproc

    # ---- x cast: fp32 -> bf16 DRAM scratch (SWDGE), per m-tile.
    # First NPRE casts lead the gpsimd ring; w loads q2/q3 follow; the
    # remaining casts trail (they only need to stay ~4 tiles ahead of PE).
    x_bf_d = dram_pool.tile([M, K], bf16, tag="x_bf", name="x_bf")

    def emit_cast(mt):
        nc.gpsimd.dma_start(
            x_bf_d[mt * 128 : (mt + 1) * 128, :], x_d[mt * 128 : (mt + 1) * 128, :]
        )

    def emit_xt(mt):
        # xT transposes ride the sync ring (scalar ring is w-transpose only)
        xT = xt_pool.tile([128, KT, 128], bf16, tag="xT", name=f"xT{mt}")
        nc.sync.dma_start(
            xT[:], x_bf_d[mt * 128 : (mt + 1) * 128, :], transpose=True
        )
        return xT

    for mt in range(NPRE):
        emit_cast(mt)
    xt_q = [emit_xt(mt) for mt in range(NPRE)]

    # ---- w preproc: int32 -> bf16 * scale -> SBUF->SBUF xbar transpose ----
    # wT_banks[b][p, k, j] = w_sc[b*512 + j, k*128 + p]
    wT_banks = [
        consts.tile([128, KT, 512], bf16, tag=f"wT{b}", name=f"wT{b}")
        for b in range(NB)
    ]
    for ot in range(OT):
        r0 = ot * 128
        w_sc4 = wq_pool.tile([128, K], bf16, tag="wsc4")
        for q in range(NWQ):
            c0 = q * WCH
            w_stage = wq_pool.tile([128, WCH], i32, tag=f"wstage{q % 2}")
            import os as _os

            if _os.environ.get("W_LOADS") == "sync":
                eng = nc.sync
            else:
                eng = nc.sync if q < 2 else nc.gpsimd
            eng.dma_start(w_stage[:], w_d[r0 : r0 + 128, c0 : c0 + WCH])
            nc.vector.tensor_scalar_mul(
                w_sc4[:, c0 : c0 + WCH], w_stage[:], scale128[:, 0:1]
            )
        b, col = ot // 4, (ot % 4) * 128
        nc.scalar.dma_start(
            wT_banks[b][:, :, col : col + 128], w_sc4[:], transpose=True
        )

    for mt in range(NPRE, MT):
        emit_cast(mt)

    # ---- main loop ----
    for mt in range(MT):
        if xt_q:
            xT = xt_q.pop(0)
        else:
            xT = emit_xt(mt)
        banks = [
            psum_pool.tile([128, 512], f32, tag=f"ps{b}", name=f"ps{mt}_{b}")
            for b in range(NB)
        ]
        for k in range(KT):
            for ob in range(NB):
                nc.tensor.matmul(
                    banks[ob][:],
                    xT[:, k, :],
                    wT_banks[ob][:, k, :],
                    start=(k == 0),
                    stop=(k == KT - 1),
                )
        for ob in range(NB):
            out_sb = out_pool.tile([128, 512], f32, tag="out", name=f"out{mt}_{ob}")
            nc.vector.tensor_add(
                out_sb[:], banks[ob][:], bias_bcast[:, ob * 512 : (ob + 1) * 512]
            )
            nc.sync.dma_start(
                y_d[mt * 128 : (mt + 1) * 128, ob * 512 : (ob + 1) * 512], out_sb[:]
            )

    ctx.close()


_CACHED_NC = None
LAST_RESULT = None


def _build_full_nc():
    global _CACHED_NC
    if _CACHED_NC is not None:
        return _CACHED_NC
    import concourse.tile as tile
    from concourse import bacc

    nc = bacc.Bacc(
        "TRN2",
        target_bir_lowering=False,
        debug=False,
        num_devices=NCORES,
    )
    with tile.TileContext(nc) as tc:
        build_kernel_v2(nc, tc, M_FULL, IN_F, O_SHARD)
    nc.compile()
    import os

    # NOTE: LDWEIGHTS dedup measured 3x SLOWER on HW (829 vs 278 ns/MM) --
    # the PE path needs the LDW/MM pairing. Keep redundant LDWs.
    if os.environ.get("DO_DEDUP") == "1":
        n = dedup_ldweights(nc)
        print(f"dedup_ldweights removed {n} instructions", file=sys.stderr)
    _CACHED_NC = nc
    return nc


def kernel(x, w_q, scale, bias):
    """Full inputs in, full output out. Shards w_q/bias over 8 cores."""
    from concourse.bass_utils import run_bass_kernel_spmd

    nc = _build_full_nc()

    x2 = np.ascontiguousarray(np.asarray(x, dtype=np.float32).reshape(M_FULL, IN_F))
    w2 = np.ascontiguousarray(np.asarray(w_q, dtype=np.int32))
    sc = np.asarray(scale, dtype=np.float32).reshape(1)
    bi = np.asarray(bias, dtype=np.float32)

    in_maps = []
    for c in range(NCORES):
        o0 = c * O_SHARD
        in_maps.append(
            {
                "x": x2,
                "w_q": np.ascontiguousarray(w2[o0 : o0 + O_SHARD]),
                "scale": sc,
                "bias": np.ascontiguousarray(bi[o0 : o0 + O_SHARD]),
            }
        )

    res = run_bass_kernel_spmd(nc, in_maps, core_ids=list(range(NCORES)))
    global LAST_RESULT
    LAST_RESULT = res
    shards = [res.results[c]["y"] for c in range(NCORES)]
    y = np.concatenate(shards, axis=1).reshape(B, S, OUT_F)
    return y.astype(np.float32)


# revision 18
# speedup vs baseline: 1.0756x; 1.0713x over previous
"""Batched compressed linear: y = x @ (w_q * scale).T + bias on 8 TRN2 cores.

Sharding: column-parallel over out_features (16384 -> 8 x 2048).
Each core computes y_shard[8192, 2048] = x[8192, 4096] @ wT_shard + bias_shard.

Design (HW-measured on trn2, see git of /root/problem for the journey):
  - Main loop is k-outer / bank-inner ("quad"): the 4 psum banks of an
    m-tile accumulate in lockstep over k, so consecutive matmuls hit
    different banks and reuse the same stationary xT tile. Measured
    278 ns/MM vs 307 ns/MM for the naive k-inner order (N=512 floor is
    213 ns; the rest is per-MM LDWEIGHTS, which this toolchain emits 1:1
    with matmuls and which must NOT be deduped: removing redundant
    LDWEIGHTS measured 3x slower).
  - scale is folded into w at preproc (w_sc = bf16(w_q)*scale), so the
    psum evict is a single DVE bias-add (rel err ~0.23%, gate is 2e-2).
  - w preproc: int32 staged [128,1024] chunks (loads split across the
    sync and gpsimd rings), DVE tensor_scalar_mul to bf16, SBUF->SBUF
    xbar transpose per 128-row strip into 4 resident k-major banks
    (16 MB SBUF). No DRAM round trip.
  - x: per-m-tile SWDGE cast DMA to a bf16 DRAM scratch, then xbar
    transpose DRAM->SBUF into k-major xT tiles on the sync ring (the
    scalar ring carries only the 16 one-time w transposes, so x
    transposes never queue behind them).
"""

import sys

if "/opt/trn_rl_repo" not in sys.path:
    sys.path.insert(0, "/opt/trn_rl_repo")

import numpy as np

B, S, IN_F, OUT_F = 4, 2048, 4096, 16384
NCORES = 8
O_SHARD = OUT_F // NCORES  # 2048
M_FULL = B * S  # 8192


def dedup_ldweights(nc):
    """Remove back-to-back InstLdweights with identical operands and no
    semaphore waits. The PE array keeps the stationary operand across
    matmuls, so a reload of the same weights is pure overhead (~107ns)."""
    import concourse.mybir as mybir

    total_removed = 0
    for blk in nc.m.functions[0].blocks:
        insts = list(blk.instructions)
        new = []
        last_key = None
        changed = False
        for i in insts:
            tn = type(i).__name__
            if tn == "InstLdweights":
                c = i.concise()
                key = (c[c.find("in=") :], str(i.tile_size), str(i.tile_position))
                has_wait = "wait:" in c
                if key == last_key and not has_wait:
                    total_removed += 1
                    changed = True
                    continue
                last_key = key
            elif tn in ("InstMatmult", "InstEventSemaphore"):
                pass  # don't invalidate the loaded stationary
            else:
                if getattr(i, "engine", None) == mybir.EngineType.PE:
                    last_key = None
            new.append(i)
        if changed:
            blk.instructions = new
    return total_removed


def build_kernel_v2(nc, tc, M, K, O):
    import concourse.mybir as mybir

    f32 = mybir.dt.float32
    bf16 = mybir.dt.bfloat16
    i32 = mybir.dt.int32

    assert M % 128 == 0 and K % 1024 == 0 and O % 512 == 0
    KT = K // 128  # 32 contraction tiles
    MT = M // 128  # 64 m tiles
    NB = O // 512  # 4 psum-bank columns
    OT = O // 128  # 16 w row-chunks
    WCH = 1024  # w free-dim chunk for staging
    NWQ = K // WCH  # 4 chunks per ot

    x_d = nc.dram_tensor("x", [M, K], f32, kind="ExternalInput").ap()
    w_d = nc.dram_tensor("w_q", [O, K], i32, kind="ExternalInput").ap()
    scale_d = nc.dram_tensor("scale", [1], f32, kind="ExternalInput").ap()
    bias_d = nc.dram_tensor("bias", [O], f32, kind="ExternalInput").ap()
    y_d = nc.dram_tensor("y", [M, O], f32, kind="ExternalOutput").ap()

    from contextlib import ExitStack

    ctx = ExitStack()
    tc_pool = lambda **kw: ctx.enter_context(tc.tile_pool(**kw))

    consts = tc_pool(name="consts", bufs=1)
    wq_pool = tc_pool(name="wq", bufs=2)
    xt_pool = tc_pool(name="xt", bufs=3)
    out_pool = tc_pool(name="outsb", bufs=4)
    psum_pool = tc_pool(name="psum", bufs=2, space="PSUM")
    dram_pool = tc_pool(name="dram", bufs=1, space="DRAM")

    # ---- constants ----
    scale128 = consts.tile([128, 1], f32, tag="scale128")
    nc.sync.dma_start(scale128[:], scale_d[None, :].partition_broadcast(128))
    bias_bcast = consts.tile([128, O], f32, tag="bias_bcast")
    nc.sync.dma_start(bias_bcast[:], bias_d[None, :].partition_broadcast(128))

    NPRE = 3  # x tiles pre-transposed ahead of the w preproc

    # ---- x cast: fp32 -> bf16 DRAM scratch (SWDGE), per m-tile.
    # First NPRE casts lead the gpsimd ring; w loads q2/q3 follow; the
    # remaining casts trail (they only need to stay ~4 tiles ahead of PE).
    x_bf_d = dram_pool.tile([M, K], bf16, tag="x_bf", name="x_bf")

    def emit_cast(mt):
        nc.gpsimd.dma_start(
            x_bf_d[mt * 128 : (mt + 1) * 128, :], x_d[mt * 128 : (mt + 1) * 128, :]
        )

    def emit_xt(mt):
        # xT transposes ride the sync ring (scalar ring is w-transpose only)
        xT = xt_pool.tile([128, KT, 128], bf16, tag="xT", name=f"xT{mt}")
        nc.sync.dma_start(
            xT[:], x_bf_d[mt * 128 : (mt + 1) * 128, :], transpose=True
        )
        return xT

    for mt in range(NPRE):
        emit_cast(mt)
    xt_q = [emit_xt(mt) for mt in range(NPRE)]

    # ---- w preproc: int32 -> bf16 * scale -> SBUF->SBUF xbar transpose ----
    # wT_banks[b][p, k, j] = w_sc[b*512 + j, k*128 + p]
    wT_banks = [
        consts.tile([128, KT, 512], bf16, tag=f"wT{b}", name=f"wT{b}")
        for b in range(NB)
    ]
    for ot in range(OT):
        r0 = ot * 128
        w_sc4 = wq_pool.tile([128, K], bf16, tag="wsc4")
        for q in range(NWQ):
            c0 = q * WCH
            w_stage = wq_pool.tile([128, WCH], i32, tag=f"wstage{q % 2}")
            # split the 33.5MB of w loads across the sync + gpsimd rings
            eng = nc.sync if q < 2 else nc.gpsimd
            eng.dma_start(w_stage[:], w_d[r0 : r0 + 128, c0 : c0 + WCH])
            nc.vector.tensor_scalar_mul(
                w_sc4[:, c0 : c0 + WCH], w_stage[:], scale128[:, 0:1]
            )
        b, col = ot // 4, (ot % 4) * 128
        nc.scalar.dma_start(
            wT_banks[b][:, :, col : col + 128], w_sc4[:], transpose=True
        )

    for mt in range(NPRE, MT):
        emit_cast(mt)

    # ---- main loop ----
    for mt in range(MT):
        if xt_q:
            xT = xt_q.pop(0)
        else:
            xT = emit_xt(mt)
        banks = [
            psum_pool.tile([128, 512], f32, tag=f"ps{b}", name=f"ps{mt}_{b}")
            for b in range(NB)
        ]
        for k in range(KT):
            for ob in range(NB):
                nc.tensor.matmul(
                    banks[ob][:],
                    xT[:, k, :],
                    wT_banks[ob][:, k, :],
                    start=(k == 0),
                    stop=(k == KT - 1),
                )
        for ob in range(NB):
            out_sb = out_pool.tile([128, 512], f32, tag="out", name=f"out{mt}_{ob}")
            nc.vector.tensor_add(
                out_sb[:], banks[ob][:], bias_bcast[:, ob * 512 : (ob + 1) * 512]
            )
            nc.sync.dma_start(
                y_d[mt * 128 : (mt + 1) * 128, ob * 512 : (ob + 1) * 512], out_sb[:]
            )

    ctx.close()


_CACHED_NC = None
LAST_RESULT = None


def _build_full_nc():
    global _CACHED_NC
    if _CACHED_NC is not None:
        return _CACHED_NC
    import concourse.tile as tile
    from concourse import bacc

    nc = bacc.Bacc(
        "TRN2",
        target_bir_lowering=False,
        debug=False,
        num_devices=NCORES,
    )
    with tile.TileContext(nc) as tc:
        build_kernel_v2(nc, tc, M_FULL, IN_F, O_SHARD)
    nc.compile()
    import os

    # NOTE: LDWEIGHTS dedup measured 3x SLOWER on HW (829 vs 278 ns/MM) --
    # the PE path needs the LDW/MM pairing. Keep redundant LDWs.
    if os.environ.get("DO_DEDUP") == "1":
        n = dedup_ldweights(nc)
        print(f"dedup_ldweights removed {n} instructions", file=sys.stderr)
    _CACHED_NC = nc
    return nc


def kernel(x, w_q, scale, bias):
    """Full inputs in, full output out. Shards w_q/bias over 8 cores."""
    from concourse.bass_utils import run_bass_kernel_spmd

    nc = _build_full_nc()

    x2 = np.ascontiguousarray(np.asarray(x, dtype=np.float32).reshape(M_FULL, IN_F))
    w2 = np.ascontiguousarray(np.asarray(w_q, dtype=np.int32))
    sc = np.asarray(scale, dtype=np.float32).reshape(1)
    bi = np.asarray(bias, dtype=np.float32)

    in_maps = []
    for c in range(NCORES):
        o0 = c * O_SHARD
        in_maps.append(
            {
                "x": x2,
                "w_q": np.ascontiguousarray(w2[o0 : o0 + O_SHARD]),
                "scale": sc,
                "bias": np.ascontiguousarray(bi[o0 : o0 + O_SHARD]),
            }
        )

    res = run_bass_kernel_spmd(nc, in_maps, core_ids=list(range(NCORES)))
    global LAST_RESULT
    LAST_RESULT = res
    shards = [res.results[c]["y"] for c in range(NCORES)]
    y = np.concatenate(shards, axis=1).reshape(B, S, OUT_F)
    return y.astype(np.float32)
